# Initial kernel scaffold
#
"""Self-contained Trainium2 Bass kernel for the AttentionBlock problem.

Shapes (hardcoded): x [8, 256, 64, 64] fp32, Wq/Wk [32, 256], bq/bk [32],
Wv [256, 256], bv [256], gamma [1].

Sharding: data-parallel over batch - each of the 8 NeuronCores computes the
full 4096x4096 attention for one batch element.  No collectives.

Per-core algorithm (C=256, C8=32, N=4096), fully SBUF-resident.
Pipeline iteration = a PAIR of groups (4 key tiles x 512-query window):
  QK   the four K=32 bf16 matmuls of both groups run as ONE 4-way
       row-packed burst (tile_position 0/32/64/96, q4/k4 replicated x4
       across partition groups) into the two [128,2,512] pair psum tiles
       - every packed output lands bank-aligned (required: packing into
       1KB-offset psum slots hangs the device), and all four stream
       concurrently.  This halves the full-row/32-row array transition
       overhead vs one pair per visit.  The QK pool has THREE buffers so
       iteration k+1's quad never waits for BOTH exps of iteration k
       (with 2 buffers the pipeline serializes through the ACT at
       2641 ns/iteration; with 3 it runs at the PE rate ~2160).
  exp  two FD=1024 ACT instructions (one per pair tile) -> pt bf16
  acc  acc += pt on DVE (bf16 2x) - per-partition rowsum partials
  AV   8 bf16 matmuls (N=512 moving keeps them streaming-bound at 215 ns;
       shorter moving lengths are LDWEIGHTS-bound) accumulate v.T@p into
       per-window av tiles [128,512] x2 (single banks, 3-buffer pool;
       DVE copies them out at window end to free banks promptly).
Per 512-query window (16 groups): rowsum = ones_g.T @ acc written into the
current iteration's first (already exp-drained) QK psum tile - its next
writer is 1.5 iterations away, past the reciprocal read - then rinv =
recip(rowsum), out = av*rinv + (gamma*bv + x).
gamma is folded into the rowsum stationary (ones_g = (1/gamma) * ones) so
the critical v-path never waits on the slow gamma broadcast DMA.  q/k
projections run as one fused chain (wq|wk in a 64-wide stationary); the
x4 replication is DVE copies for the two pre-pipeline windows and
SBUF-to-SBUF DMAs on idle queues for dripped windows.  PSUM: QK 3x2
banks + AV 2x1 = 8 exactly.

Startup lessons baked in: only sync + scalar queues are hardware-DGE
(gpsimd SWDGE transfers have ~10 us latency); every DMA dependency hop
costs ~3-8 us of completion latency, so window 0's x rides dedicated
priority DMAs and nothing on the critical path consumes the tiny
4-byte-packet bias/gamma loads; Tile schedules by dependency, not
emission order, so the DVE FIFO must not be gated on slow DMAs.  The
chip has a ~1.2x power-throttle state - compare runs via the exp
ACTIVATE duration (1114 ns full clock).
"""

import sys

import numpy as np

if "/opt/trn_rl_repo" not in sys.path:
    sys.path.insert(0, "/opt/trn_rl_repo")

import concourse.bass as bass
import concourse.bacc as bacc
import concourse.tile as tile
from concourse import mybir
from concourse.bass_utils import run_bass_kernel_spmd
from concourse.masks import make_identity

F32 = mybir.dt.float32
BF16 = mybir.dt.bfloat16

C = 256
C8 = 32
P = 128
CH = C // P  # 2 channel chunks
IW = 512     # query-window size


def build_attention_nc(n: int = 4096) -> bass.Bass:
    """Build the single-core Bass program (SPMD across 8 cores)."""
    assert n % IW == 0
    NW = n // IW        # query windows (8)
    JT = n // P         # key tiles (32)
    GPW = JT // 2       # groups per window (16)
    NG = NW * GPW       # total groups (128)
    NH = n // 2         # half of the token dim (x loaded as 2 halves)

    nc = bacc.Bacc("TRN2", target_bir_lowering=False)
    x_d = nc.declare_dram_parameter("x", [C, n], F32, isOutput=False)
    wq_d = nc.declare_dram_parameter("Wq", [C8, C], F32, isOutput=False)
    bq_d = nc.declare_dram_parameter("bq", [C8], F32, isOutput=False)
    wk_d = nc.declare_dram_parameter("Wk", [C8, C], F32, isOutput=False)
    bk_d = nc.declare_dram_parameter("bk", [C8], F32, isOutput=False)
    wv_d = nc.declare_dram_parameter("Wv", [C, C], F32, isOutput=False)
    bv_d = nc.declare_dram_parameter("bv", [C], F32, isOutput=False)
    gamma_d = nc.declare_dram_parameter("gamma", [1], F32, isOutput=False)
    out_d = nc.declare_dram_parameter("out", [C, n], F32, isOutput=True)

    with tile.TileContext(nc) as tc:
        with (
            tc.tile_pool(name="const", bufs=1) as const,
            tc.tile_pool(name="xpool", bufs=1) as xpool,
            tc.tile_pool(name="qkpool", bufs=1) as qkpool,
            tc.tile_pool(name="vtpool", bufs=1) as vtpool,
            tc.tile_pool(name="ptpool", bufs=6) as ptpool,
            tc.tile_pool(name="accpool", bufs=3) as accpool,
            tc.tile_pool(name="smallwork", bufs=4) as smallwork,
            tc.tile_pool(name="outpool", bufs=8) as outpool,
            tc.tile_pool(name="pe_ps", bufs=3, space="PSUM") as pe_ps,  # 3x2 banks
            tc.tile_pool(name="av_ps", bufs=2, space="PSUM") as av_ps,  # 2x1 banks
        ):
            # ---------------- setup: loads ----------------
            ident = const.tile([P, P], F32, tag="ident")
            make_identity(nc, ident)

            ones_bf = const.tile([P, P], BF16, tag="ones")
            nc.vector.memset(ones_bf, 1.0)
            ones_g = const.tile([P, P], BF16, tag="onesg")

            # x loads in quarters.  Both HWDGE queues (sync + scalar) carry
            # them - the gpsimd SWDGE path has ~10us transfer latency and is
            # avoided for anything startup-critical.  Weights go first on
            # sync (they gate the transposes); the early x ch1 quarters ride
            # the scalar queue which is otherwise idle until the first exp.
            NQT = NH // 2
            xq = [xpool.tile([P, CH, NQT], F32, tag=f"xq{i}", name=f"xq{i}")
                  for i in range(4)]
            xbq = [xpool.tile([P, CH, NQT], BF16, tag=f"xbq{i}", name=f"xbq{i}")
                   for i in range(4)]
            # priority copies of window 0's x so the projection chain can
            # start ~5us before the bulk quarters land
            x0 = xpool.tile([P, CH, IW], F32, tag="x0")
            xb0 = xpool.tile([P, CH, IW], BF16, tag="xb0")
            wq_stage = const.tile([C8, C], F32, tag="wqs")
            nc.sync.dma_start(out=wq_stage, in_=wq_d[:, :])
            nc.sync.dma_start(out=x0[:, 0, :], in_=x_d[0:P, 0:IW])
            nc.scalar.dma_start(out=x0[:, 1, :], in_=x_d[P : 2 * P, 0:IW])
            # warm the ACT exp table (after the critical DMA descriptors)
            warm_in = const.tile([P, 1], F32, tag="warmin")
            nc.gpsimd.memset(warm_in, 0.0)
            warm_out = const.tile([P, 1], F32, tag="warmout")
            nc.scalar.activation(warm_out, warm_in, mybir.ActivationFunctionType.Exp)
            wk_stage = const.tile([C8, C], F32, tag="wks")
            nc.sync.dma_start(out=wk_stage, in_=wk_d[:, :])
            wv_stage = const.tile([P, CH, C], F32, tag="wvs")
            nc.sync.dma_start(
                out=wv_stage, in_=wv_d[:, :].rearrange("(a p) c -> p a c", p=P)
            )
            nc.scalar.dma_start(out=xq[0][:, 1, :], in_=x_d[P : 2 * P, 0:NQT])
            nc.scalar.dma_start(
                out=xq[1][:, 1, :], in_=x_d[P : 2 * P, NQT : 2 * NQT]
            )
            bq_sb = const.tile([C8, 1], F32, tag="bq")
            nc.scalar.dma_start(
                out=bq_sb, in_=bq_d[:].rearrange("(p one) -> p one", one=1)
            )
            bk_sb = const.tile([C8, 1], F32, tag="bk")
            nc.scalar.dma_start(
                out=bk_sb, in_=bk_d[:].rearrange("(p one) -> p one", one=1)
            )
            bv2_sb = const.tile([CH, P], F32, tag="bv2")
            nc.scalar.dma_start(
                out=bv2_sb, in_=bv_d[:].rearrange("(ch p) -> ch p", p=P)
            )
            gamma_ap = gamma_d[:]
            gamma_sb = const.tile([P, 1], F32, tag="gamma")
            nc.scalar.dma_start(
                out=gamma_sb,
                in_=bass.AP(
                    tensor=gamma_ap.tensor, offset=gamma_ap.offset,
                    ap=[[0, P], gamma_ap.ap[0]],
                ),
            )
            for i in range(4):
                lo = i * NQT
                nc.sync.dma_start(out=xq[i][:, 0, :], in_=x_d[0:P, lo : lo + NQT])
            nc.sync.dma_start(
                out=xq[2][:, 1, :], in_=x_d[P : 2 * P, 2 * NQT : 3 * NQT]
            )
            nc.sync.dma_start(
                out=xq[3][:, 1, :], in_=x_d[P : 2 * P, 3 * NQT : 4 * NQT]
            )
            gbv = const.tile([P, CH], F32, tag="gbv")

            def x_win(iw):  # fp32 residual slice [P, CH, IW]
                if iw == 0:
                    return x0[:, :, :]
                i = (iw * IW) // NQT
                off = iw * IW - i * NQT
                return xq[i][:, :, off : off + IW]

            def xb_win(iw):  # bf16 slice [P, CH, IW]
                if iw == 0:
                    return xb0[:, :, :]
                i = (iw * IW) // NQT
                off = iw * IW - i * NQT
                return xbq[i][:, :, off : off + IW]

            def emit_xcast(iw):
                nc.vector.tensor_copy(xb_win(iw), x_win(iw))

            # ------------- weight transposes (bf16) -------------
            # wqkt[c, ch, 0:32] = wq^T chunk, wqkt[c, ch, 32:64] = wk^T chunk,
            # so one matmul chain projects q and k together.
            wqkt = const.tile([P, CH, 2 * C8], BF16, tag="wqkt")
            for ch in range(CH):
                ps_tqk = pe_ps.tile([P, 2 * C8], F32, tag="peps", name=f"ps_tqk{ch}")
                nc.tensor.transpose(
                    ps_tqk[:, 0:C8], wq_stage[:, bass.ts(ch, P)], ident[:C8, :C8]
                )
                nc.tensor.transpose(
                    ps_tqk[:, C8 : 2 * C8], wk_stage[:, bass.ts(ch, P)],
                    ident[:C8, :C8]
                )
                nc.vector.tensor_copy(wqkt[:, ch, :], ps_tqk)

            emit_xcast(0)

            # wvt[c, ci, o] = Wv[o, ci*128+c], bf16 (gamma is folded into
            # the rowsum stationary ones_g = (1/gamma) * ones instead)
            wvt = const.tile([P, CH, C], BF16, tag="wvt")
            for ci in range(CH):
                for oi in range(CH):
                    pool, ptag = (pe_ps, "peps") if oi == 0 else (av_ps, "avps")
                    ps_tv = pool.tile([P, P], F32, tag=ptag, name=f"ps_tv{ci}{oi}")
                    nc.tensor.transpose(
                        ps_tv, wv_stage[:, oi, bass.ts(ci, P)], ident
                    )
                    nc.vector.tensor_copy(wvt[:, ci, bass.ts(oi, P)], ps_tv)

            # ---------------- projections ----------------
            # q4/k4: [64, n] bf16, q/k replicated x2 across partition groups
            # for the 2-way row-packed QK matmuls.  One fused chain projects
            # q and k together into qk_s; idle DMA queues do the replication.
            qk_s = qkpool.tile([2 * C8, n], BF16, tag="qks")
            q4 = qkpool.tile([4 * C8, n], BF16, tag="q4")
            k4 = qkpool.tile([4 * C8, n], BF16, tag="k4")
            bqk_sb = const.tile([2 * C8, 1], F32, tag="bqk")

            def emit_qkproj(iw, startup=False):
                win = bass.ts(iw, IW)
                xbw = xb_win(iw)
                ps_qk = pe_ps.tile([P, IW], F32, tag="peps", name=f"ps_qk_{iw}")
                for ch in range(CH):
                    nc.tensor.matmul(
                        ps_qk[0 : 2 * C8, :], wqkt[:, ch, :], xbw[:, ch, :],
                        start=(ch == 0), stop=(ch == CH - 1),
                    )
                if startup:
                    # pre-pipeline: DVE is idle and has far lower latency
                    # than the ACT queue + DMA hops
                    nc.vector.tensor_scalar_add(
                        qk_s[:, win], ps_qk[0 : 2 * C8, :], bqk_sb
                    )
                    for r in range(4):
                        nc.vector.tensor_copy(
                            q4[r * C8 : (r + 1) * C8, win], qk_s[0:C8, win]
                        )
                        nc.vector.tensor_copy(
                            k4[r * C8 : (r + 1) * C8, win], qk_s[C8 : 2 * C8, win]
                        )
                else:
                    nc.scalar.activation(
                        qk_s[:, win], ps_qk[0 : 2 * C8, :],
                        mybir.ActivationFunctionType.Identity,
                        bias=bqk_sb, scale=1.0,
                    )
                    for r in range(4):
                        eng = nc.sync if r % 2 == 0 else nc.gpsimd
                        eng.dma_start(
                            out=q4[r * C8 : (r + 1) * C8, win], in_=qk_s[0:C8, win]
                        )
                        eng2 = nc.gpsimd if r % 2 == 0 else nc.sync
                        eng2.dma_start(
                            out=k4[r * C8 : (r + 1) * C8, win],
                            in_=qk_s[C8 : 2 * C8, win],
                        )

            # vT per key tile: vt[jt][p, c] = gamma * (Wv x)[c, jt*128+p], bf16
            vt = [None] * JT

            def emit_vproj(jt, cast_on_act=False):
                vtt = vtpool.tile([P, C], BF16, tag=f"vt{jt}", name=f"vt{jt}")
                ps_v = pe_ps.tile([P, C], F32, tag="peps", name=f"ps_v{jt}")
                iww, off = (jt * P) // IW, (jt * P) % IW
                xbw = xb_win(iww)
                for ch in range(CH):
                    nc.tensor.matmul(
                        ps_v,
                        xbw[:, ch, off : off + P],
                        wvt[:, ch, :],
                        start=(ch == 0), stop=(ch == CH - 1),
                    )
                if cast_on_act:
                    nc.scalar.copy(vtt, ps_v)
                else:
                    nc.vector.tensor_copy(vtt, ps_v)
                vt[jt] = vtt

            nc.vector.tensor_copy(bqk_sb[0:C8, :], bq_sb)
            nc.vector.tensor_copy(bqk_sb[C8 : 2 * C8, :], bk_sb)
            emit_qkproj(0, startup=True)
            for jt in range(4):
                emit_vproj(jt)
            emit_xcast(1)
            emit_qkproj(1, startup=True)
            for jt in range(4, 8):
                emit_vproj(jt)
            emit_xcast(2)
            vjt_late = list(range(8, JT))
            qk_late = list(range(2, NW))
            xc_late = list(range(3, NW))

            def emit_consts():
                # gbv = gamma * bv via on-chip transpose of the fast-shape
                # load; rowsum stationary carries 1/gamma -> rinv = g/rowsum
                ps_bv = pe_ps.tile([P, CH], F32, tag="peps", name="ps_bv")
                nc.tensor.transpose(ps_bv, bv2_sb, ident[:CH, :CH])
                nc.vector.tensor_scalar_mul(gbv, ps_bv, gamma_sb)
                giv = const.tile([P, 1], F32, tag="giv")
                nc.vector.reciprocal(giv, gamma_sb)
                nc.vector.tensor_scalar_mul(ones_g, ones_bf, giv)

            # ---------------- main pipeline ----------------
            # Iterate over PAIRS of groups: the four K=32 QK matmuls of two
            # consecutive groups run as one 4-way row-packed burst
            # (tile_position 0/32/64/96) into the two pair psum tiles - all
            # slots bank-aligned, all four streaming concurrently.  This
            # halves the full-row/32-row array transition overhead.
            state = {}
            last_tiles = [None]

            def emit_quad(k):
                g0 = 2 * k
                pts = []
                tiles = []
                for j, g in enumerate((g0, g0 + 1)):
                    iw, gg = divmod(g, GPW)
                    if gg == 0:
                        state[iw] = {
                            "av": [
                                av_ps.tile([P, IW], F32, tag="avps",
                                           name=f"av{c}_{iw}")
                                for c in range(CH)
                            ],
                            "acc": accpool.tile(
                                [P, 2, IW], BF16, tag="acc", name=f"acc_{iw}"
                            ),
                        }
                    tiles.append(pe_ps.tile([P, 2, IW], F32, tag="peps",
                                            name=f"ps_e{g}"))
                for j, g in enumerate((g0, g0 + 1)):
                    iw, gg = divmod(g, GPW)
                    win = bass.ts(iw, IW)
                    for m in range(2):
                        jt = 2 * gg + m
                        r = 2 * j + m
                        nc.tensor.matmul(
                            tiles[j][:, m, :],
                            k4[r * C8 : (r + 1) * C8, bass.ts(jt, P)],
                            q4[r * C8 : (r + 1) * C8, win],
                            start=True, stop=True,
                            tile_position=(r * C8, 0),
                        )
                last_tiles[0] = tiles
                for j, g in enumerate((g0, g0 + 1)):
                    iw, gg = divmod(g, GPW)
                    pt = ptpool.tile([P, 2, IW], BF16, tag="pt", name=f"pt{g}")
                    nc.scalar.activation(
                        pt, tiles[j], mybir.ActivationFunctionType.Exp
                    )
                    acc = state[iw]["acc"]
                    if gg == 0:
                        nc.vector.tensor_copy(acc, pt)
                    else:
                        nc.vector.tensor_add(acc, acc, pt)
                    pts.append(pt)
                return pts

            def emit_av(g, pt):
                iw, gg = divmod(g, GPW)
                av = state[iw]["av"]
                for m in range(2):
                    jt = 2 * gg + m
                    for ch in range(CH):
                        nc.tensor.matmul(
                            av[ch],
                            vt[jt][:, bass.ts(ch, P)],
                            pt[:, m, :],
                            start=(gg == 0 and m == 0),
                            stop=(gg == GPW - 1 and m == 1),
                            skip_group_check=True,
                        )
                if gg == GPW - 1:
                    # free the av banks promptly: one copy on the DVE, one
                    # on the scalar engine so the two drain in parallel and
                    # the next window's AV matmuls start ~700ns sooner
                    av_sb = []
                    for ch in range(CH):
                        a_sb = outpool.tile([P, IW], F32, tag="osb",
                                            name=f"avsb{ch}_{iw}")
                        if ch == 0:
                            nc.vector.tensor_copy(a_sb, av[ch])
                        else:
                            nc.scalar.copy(a_sb, av[ch])
                        av_sb.append(a_sb)
                    state[iw]["av_sb"] = av_sb

            def emit_epilogue(iw):
                st = state.pop(iw)
                acc, av_sb = st["acc"], st["av_sb"]
                win = bass.ts(iw, IW)
                ps_r = last_tiles[0][0][:, 0, :]
                for s in range(2):
                    nc.tensor.matmul(
                        ps_r, ones_g, acc[:, s, :],
                        start=(s == 0), stop=(s == 1),
                    )
                rinv = smallwork.tile([P, IW], F32, tag="rinv", name=f"rinv{iw}")
                nc.vector.reciprocal_approx_fast(rinv, ps_r)
                xw = x_win(iw)
                for ch in range(CH):
                    o_sb = outpool.tile([P, IW], F32, tag="osb",
                                        name=f"osb{ch}_{iw}")
                    nc.vector.tensor_mul(o_sb, av_sb[ch], rinv)
                    nc.vector.scalar_tensor_tensor(
                        out=o_sb, in0=o_sb, scalar=gbv[:, ch : ch + 1],
                        in1=xw[:, ch, :],
                        op0=mybir.AluOpType.add, op1=mybir.AluOpType.add,
                    )
                    if ch == 0:
                        eng = nc.sync
                    elif iw == NW - 1:
                        eng = nc.scalar
                    else:
                        eng = nc.gpsimd
                    eng.dma_start(
                        out=out_d[ch * P : (ch + 1) * P, win], in_=o_sb
                    )

            NK = NG // 2
            prev = None
            for k in range(NK + 1):
                if k < NK:
                    if xc_late:
                        emit_xcast(xc_late.pop(0))
                    cur = emit_quad(k)
                if k > 0:
                    emit_av(2 * (k - 1), prev[0])
                    emit_av(2 * (k - 1) + 1, prev[1])
                if k < NK:
                    if qk_late:
                        emit_qkproj(qk_late.pop(0))
                    for _ in range(4):
                        if vjt_late:
                            emit_vproj(vjt_late.pop(0))
                    if k == 2:
                        emit_consts()
                    prev = cur
                g_done = 2 * (k - 1) + 1 if k > 0 else -1
                for w in range(NW):
                    if w in state and "av_sb" in state[w] and g_done >= 16 * w + 17:
                        emit_epilogue(w)
            for w in range(NW):
                if w in state:
                    emit_epilogue(w)

    nc.finalize()
    return nc


_NC_CACHE: dict[int, bass.Bass] = {}


def _get_nc(n: int) -> bass.Bass:
    if n not in _NC_CACHE:
        _NC_CACHE[n] = build_attention_nc(n)
    return _NC_CACHE[n]


def kernel(x, Wq, bq, Wk, bk, Wv, bv, gamma):
    B, c, h, w = x.shape
    n = h * w
    assert B == 8 and c == C
    nc = _get_nc(n)
    xf = np.ascontiguousarray(np.asarray(x, dtype=np.float32).reshape(B, c, n))
    common = {
        "Wq": np.ascontiguousarray(np.asarray(Wq, dtype=np.float32)),
        "bq": np.ascontiguousarray(np.asarray(bq, dtype=np.float32)),
        "Wk": np.ascontiguousarray(np.asarray(Wk, dtype=np.float32)),
        "bk": np.ascontiguousarray(np.asarray(bk, dtype=np.float32)),
        "Wv": np.ascontiguousarray(np.asarray(Wv, dtype=np.float32)),
        "bv": np.ascontiguousarray(np.asarray(bv, dtype=np.float32)),
        "gamma": np.ascontiguousarray(np.asarray(gamma, dtype=np.float32)),
    }
    in_maps = [{"x": xf[b], **common} for b in range(B)]
    res = run_bass_kernel_spmd(nc, in_maps, core_ids=list(range(B)))
    out = np.stack([res.results[b]["out"].reshape(c, h, w) for b in range(B)])
    return out.astype(np.float32)



# revision 1
# speedup vs baseline: 1.0931x; 1.0931x over previous
"""Self-contained Trainium2 Bass kernel for the AttentionBlock problem.

Shapes (hardcoded): x [8, 256, 64, 64] fp32, Wq/Wk [32, 256], bq/bk [32],
Wv [256, 256], bv [256], gamma [1].

Sharding: data-parallel over batch - each of the 8 NeuronCores computes the
full 4096x4096 attention for one batch element.  No collectives.

Per-core algorithm (C=256, C8=32, N=4096), fully SBUF-resident.
Pipeline iteration = a PAIR of groups (4 key tiles x 512-query window):
  QK   the four K=32 bf16 matmuls of both groups run as ONE 4-way
       row-packed burst (tile_position 0/32/64/96, q4/k4 replicated x4
       across partition groups) into the two [128,2,512] pair psum tiles
       - every packed output lands bank-aligned (required: packing into
       1KB-offset psum slots hangs the device), and all four stream
       concurrently.  This halves the full-row/32-row array transition
       overhead vs one pair per visit.  The QK pool has THREE buffers so
       iteration k+1's quad never waits for BOTH exps of iteration k
       (with 2 buffers the pipeline serializes through the ACT at
       2641 ns/iteration; with 3 it runs at the PE rate ~2160).
  exp  two FD=1024 ACT instructions (one per pair tile) -> pt bf16
  acc  acc += pt on DVE (bf16 2x) - per-partition rowsum partials
  AV   8 bf16 matmuls (N=512 moving keeps them streaming-bound at 215 ns;
       shorter moving lengths are LDWEIGHTS-bound) accumulate v.T@p into
       per-window av tiles [128,512] x2 (single banks, 3-buffer pool;
       DVE copies them out at window end to free banks promptly).
Per 512-query window (16 groups): rowsum = ones_g.T @ acc written into the
current iteration's first (already exp-drained) QK psum tile - its next
writer is 1.5 iterations away, past the reciprocal read - then rinv =
recip(rowsum), out = av*rinv + (gamma*bv + x).
gamma is folded into the rowsum stationary (ones_g = (1/gamma) * ones) so
the critical v-path never waits on the slow gamma broadcast DMA.  q/k
projections run as one fused chain (wq|wk in a 64-wide stationary); the
x4 replication is DVE copies for the two pre-pipeline windows and
SBUF-to-SBUF DMAs on idle queues for dripped windows.  PSUM: QK 3x2
banks + AV 2x1 = 8 exactly.

Startup lessons baked in: only sync + scalar queues are hardware-DGE
(gpsimd SWDGE transfers have ~10 us latency); every DMA dependency hop
costs ~3-8 us of completion latency, so window 0's x rides dedicated
priority DMAs and nothing on the critical path consumes the tiny
4-byte-packet bias/gamma loads; Tile schedules by dependency, not
emission order, so the DVE FIFO must not be gated on slow DMAs.  The
chip has a ~1.2x power-throttle state - compare runs via the exp
ACTIVATE duration (1114 ns full clock).
"""

import sys

import numpy as np

if "/opt/trn_rl_repo" not in sys.path:
    sys.path.insert(0, "/opt/trn_rl_repo")

import concourse.bass as bass
import concourse.bacc as bacc
import concourse.tile as tile
from concourse import mybir
from concourse.bass_utils import run_bass_kernel_spmd
from concourse.masks import make_identity

F32 = mybir.dt.float32
BF16 = mybir.dt.bfloat16

C = 256
C8 = 32
P = 128
CH = C // P  # 2 channel chunks
IW = 512     # query-window size


def build_attention_nc(n: int = 4096) -> bass.Bass:
    """Build the single-core Bass program (SPMD across 8 cores)."""
    assert n % IW == 0
    NW = n // IW        # query windows (8)
    JT = n // P         # key tiles (32)
    GPW = JT // 2       # groups per window (16)
    NG = NW * GPW       # total groups (128)
    NH = n // 2         # half of the token dim (x loaded as 2 halves)

    nc = bacc.Bacc("TRN2", target_bir_lowering=False)
    x_d = nc.declare_dram_parameter("x", [C, n], F32, isOutput=False)
    wq_d = nc.declare_dram_parameter("Wq", [C8, C], F32, isOutput=False)
    bq_d = nc.declare_dram_parameter("bq", [C8], F32, isOutput=False)
    wk_d = nc.declare_dram_parameter("Wk", [C8, C], F32, isOutput=False)
    bk_d = nc.declare_dram_parameter("bk", [C8], F32, isOutput=False)
    wv_d = nc.declare_dram_parameter("Wv", [C, C], F32, isOutput=False)
    bv_d = nc.declare_dram_parameter("bv", [C], F32, isOutput=False)
    gamma_d = nc.declare_dram_parameter("gamma", [1], F32, isOutput=False)
    out_d = nc.declare_dram_parameter("out", [C, n], F32, isOutput=True)

    with tile.TileContext(nc) as tc:
        with (
            tc.tile_pool(name="const", bufs=1) as const,
            tc.tile_pool(name="xpool", bufs=1) as xpool,
            tc.tile_pool(name="qkpool", bufs=1) as qkpool,
            tc.tile_pool(name="vtpool", bufs=1) as vtpool,
            tc.tile_pool(name="ptpool", bufs=6) as ptpool,
            tc.tile_pool(name="accpool", bufs=3) as accpool,
            tc.tile_pool(name="smallwork", bufs=4) as smallwork,
            tc.tile_pool(name="outpool", bufs=8) as outpool,
            tc.tile_pool(name="pe_ps", bufs=3, space="PSUM") as pe_ps,  # 3x2 banks
            tc.tile_pool(name="av_ps", bufs=2, space="PSUM") as av_ps,  # 2x1 banks
        ):
            # ---------------- setup: loads ----------------
            ident = const.tile([P, P], F32, tag="ident")
            make_identity(nc, ident)

            ones_bf = const.tile([P, P], BF16, tag="ones")
            nc.vector.memset(ones_bf, 1.0)
            ones_g = const.tile([P, P], BF16, tag="onesg")

            # x loads in quarters.  Both HWDGE queues (sync + scalar) carry
            # them - the gpsimd SWDGE path has ~10us transfer latency and is
            # avoided for anything startup-critical.  Weights go first on
            # sync (they gate the transposes); the early x ch1 quarters ride
            # the scalar queue which is otherwise idle until the first exp.
            NQT = NH // 2
            xq = [xpool.tile([P, CH, NQT], F32, tag=f"xq{i}", name=f"xq{i}")
                  for i in range(4)]
            xbq = [xpool.tile([P, CH, NQT], BF16, tag=f"xbq{i}", name=f"xbq{i}")
                   for i in range(4)]
            # priority copies of window 0's x so the projection chain can
            # start ~5us before the bulk quarters land
            x0 = xpool.tile([P, CH, IW], F32, tag="x0")
            xb0 = xpool.tile([P, CH, IW], BF16, tag="xb0")
            wq_stage = const.tile([C8, C], F32, tag="wqs")
            nc.sync.dma_start(out=wq_stage, in_=wq_d[:, :])
            nc.sync.dma_start(out=x0[:, 0, :], in_=x_d[0:P, 0:IW])
            nc.scalar.dma_start(out=x0[:, 1, :], in_=x_d[P : 2 * P, 0:IW])
            # warm the ACT exp table (after the critical DMA descriptors)
            warm_in = const.tile([P, 1], F32, tag="warmin")
            nc.gpsimd.memset(warm_in, 0.0)
            warm_out = const.tile([P, 1], F32, tag="warmout")
            nc.scalar.activation(warm_out, warm_in, mybir.ActivationFunctionType.Exp)
            wk_stage = const.tile([C8, C], F32, tag="wks")
            nc.sync.dma_start(out=wk_stage, in_=wk_d[:, :])
            wv_stage = const.tile([P, CH, C], F32, tag="wvs")
            nc.sync.dma_start(
                out=wv_stage, in_=wv_d[:, :].rearrange("(a p) c -> p a c", p=P)
            )
            nc.scalar.dma_start(out=xq[0][:, 1, :], in_=x_d[P : 2 * P, 0:NQT])
            nc.scalar.dma_start(
                out=xq[1][:, 1, :], in_=x_d[P : 2 * P, NQT : 2 * NQT]
            )
            bq_sb = const.tile([C8, 1], F32, tag="bq")
            nc.scalar.dma_start(
                out=bq_sb, in_=bq_d[:].rearrange("(p one) -> p one", one=1)
            )
            bk_sb = const.tile([C8, 1], F32, tag="bk")
            nc.scalar.dma_start(
                out=bk_sb, in_=bk_d[:].rearrange("(p one) -> p one", one=1)
            )
            bv2_sb = const.tile([CH, P], F32, tag="bv2")
            nc.scalar.dma_start(
                out=bv2_sb, in_=bv_d[:].rearrange("(ch p) -> ch p", p=P)
            )
            gamma_ap = gamma_d[:]
            gamma_sb = const.tile([P, 1], F32, tag="gamma")
            nc.scalar.dma_start(
                out=gamma_sb,
                in_=bass.AP(
                    tensor=gamma_ap.tensor, offset=gamma_ap.offset,
                    ap=[[0, P], gamma_ap.ap[0]],
                ),
            )
            for i in range(4):
                lo = i * NQT
                nc.sync.dma_start(out=xq[i][:, 0, :], in_=x_d[0:P, lo : lo + NQT])
            nc.sync.dma_start(
                out=xq[2][:, 1, :], in_=x_d[P : 2 * P, 2 * NQT : 3 * NQT]
            )
            nc.sync.dma_start(
                out=xq[3][:, 1, :], in_=x_d[P : 2 * P, 3 * NQT : 4 * NQT]
            )
            gbv = const.tile([P, CH], F32, tag="gbv")

            def x_win(iw):  # fp32 residual slice [P, CH, IW]
                if iw == 0:
                    return x0[:, :, :]
                i = (iw * IW) // NQT
                off = iw * IW - i * NQT
                return xq[i][:, :, off : off + IW]

            def xb_win(iw):  # bf16 slice [P, CH, IW]
                if iw == 0:
                    return xb0[:, :, :]
                i = (iw * IW) // NQT
                off = iw * IW - i * NQT
                return xbq[i][:, :, off : off + IW]

            def emit_xcast(iw):
                nc.vector.tensor_copy(xb_win(iw), x_win(iw))

            # ------------- weight transposes (bf16) -------------
            # wqkt[c, ch, 0:32] = wq^T chunk, wqkt[c, ch, 32:64] = wk^T chunk,
            # so one matmul chain projects q and k together.
            wqkt = const.tile([P, CH, 2 * C8], BF16, tag="wqkt")
            for ch in range(CH):
                ps_tqk = pe_ps.tile([P, 2 * C8], F32, tag="peps", name=f"ps_tqk{ch}")
                nc.tensor.transpose(
                    ps_tqk[:, 0:C8], wq_stage[:, bass.ts(ch, P)], ident[:C8, :C8]
                )
                nc.tensor.transpose(
                    ps_tqk[:, C8 : 2 * C8], wk_stage[:, bass.ts(ch, P)],
                    ident[:C8, :C8]
                )
                nc.vector.tensor_copy(wqkt[:, ch, :], ps_tqk)

            emit_xcast(0)

            # wvt[c, ci, o] = Wv[o, ci*128+c], bf16 (gamma is folded into
            # the rowsum stationary ones_g = (1/gamma) * ones instead)
            wvt = const.tile([P, CH, C], BF16, tag="wvt")
            for ci in range(CH):
                for oi in range(CH):
                    pool, ptag = (pe_ps, "peps") if oi == 0 else (av_ps, "avps")
                    ps_tv = pool.tile([P, P], F32, tag=ptag, name=f"ps_tv{ci}{oi}")
                    nc.tensor.transpose(
                        ps_tv, wv_stage[:, oi, bass.ts(ci, P)], ident
                    )
                    nc.vector.tensor_copy(wvt[:, ci, bass.ts(oi, P)], ps_tv)

            # ---------------- projections ----------------
            # q4/k4: [64, n] bf16, q/k replicated x2 across partition groups
            # for the 2-way row-packed QK matmuls.  One fused chain projects
            # q and k together into qk_s; idle DMA queues do the replication.
            qk_s = qkpool.tile([2 * C8, n], BF16, tag="qks")
            q4 = qkpool.tile([4 * C8, n], BF16, tag="q4")
            k4 = qkpool.tile([4 * C8, n], BF16, tag="k4")
            bqk_sb = const.tile([2 * C8, 1], F32, tag="bqk")

            def emit_qkproj(iw, startup=False):
                win = bass.ts(iw, IW)
                xbw = xb_win(iw)
                ps_qk = pe_ps.tile([P, IW], F32, tag="peps", name=f"ps_qk_{iw}")
                for ch in range(CH):
                    nc.tensor.matmul(
                        ps_qk[0 : 2 * C8, :], wqkt[:, ch, :], xbw[:, ch, :],
                        start=(ch == 0), stop=(ch == CH - 1),
                    )
                if startup:
                    # pre-pipeline: DVE is idle and has far lower latency
                    # than the ACT queue + DMA hops
                    nc.vector.tensor_scalar_add(
                        qk_s[:, win], ps_qk[0 : 2 * C8, :], bqk_sb
                    )
                    for r in range(4):
                        nc.vector.tensor_copy(
                            q4[r * C8 : (r + 1) * C8, win], qk_s[0:C8, win]
                        )
                        nc.vector.tensor_copy(
                            k4[r * C8 : (r + 1) * C8, win], qk_s[C8 : 2 * C8, win]
                        )
                else:
                    nc.scalar.activation(
                        qk_s[:, win], ps_qk[0 : 2 * C8, :],
                        mybir.ActivationFunctionType.Identity,
                        bias=bqk_sb, scale=1.0,
                    )
                    for r in range(4):
                        eng = nc.sync if r % 2 == 0 else nc.gpsimd
                        eng.dma_start(
                            out=q4[r * C8 : (r + 1) * C8, win], in_=qk_s[0:C8, win]
                        )
                        eng2 = nc.gpsimd if r % 2 == 0 else nc.sync
                        eng2.dma_start(
                            out=k4[r * C8 : (r + 1) * C8, win],
                            in_=qk_s[C8 : 2 * C8, win],
                        )

            # vT per key tile: vt[jt][p, c] = gamma * (Wv x)[c, jt*128+p], bf16
            vt = [None] * JT

            def emit_vproj(jt, cast_on_act=False):
                vtt = vtpool.tile([P, C], BF16, tag=f"vt{jt}", name=f"vt{jt}")
                ps_v = pe_ps.tile([P, C], F32, tag="peps", name=f"ps_v{jt}")
                iww, off = (jt * P) // IW, (jt * P) % IW
                xbw = xb_win(iww)
                for ch in range(CH):
                    nc.tensor.matmul(
                        ps_v,
                        xbw[:, ch, off : off + P],
                        wvt[:, ch, :],
                        start=(ch == 0), stop=(ch == CH - 1),
                    )
                if cast_on_act:
                    nc.scalar.copy(vtt, ps_v)
                else:
                    nc.vector.tensor_copy(vtt, ps_v)
                vt[jt] = vtt

            nc.vector.tensor_copy(bqk_sb[0:C8, :], bq_sb)
            nc.vector.tensor_copy(bqk_sb[C8 : 2 * C8, :], bk_sb)
            emit_qkproj(0, startup=True)
            for jt in range(4):
                emit_vproj(jt)
            emit_xcast(1)
            emit_qkproj(1, startup=True)
            for jt in range(4, 8):
                emit_vproj(jt)
            emit_xcast(2)
            vjt_late = list(range(8, JT))
            qk_late = list(range(2, NW))
            xc_late = list(range(3, NW))

            def emit_consts():
                # gbv = gamma * bv via on-chip transpose of the fast-shape
                # load; rowsum stationary carries 1/gamma -> rinv = g/rowsum
                ps_bv = pe_ps.tile([P, CH], F32, tag="peps", name="ps_bv")
                nc.tensor.transpose(ps_bv, bv2_sb, ident[:CH, :CH])
                nc.vector.tensor_scalar_mul(gbv, ps_bv, gamma_sb)
                giv = const.tile([P, 1], F32, tag="giv")
                nc.vector.reciprocal(giv, gamma_sb)
                nc.vector.tensor_scalar_mul(ones_g, ones_bf, giv)

            # ---------------- main pipeline ----------------
            # Iterate over PAIRS of groups: the four K=32 QK matmuls of two
            # consecutive groups run as one 4-way row-packed burst
            # (tile_position 0/32/64/96) into the two pair psum tiles - all
            # slots bank-aligned, all four streaming concurrently.  This
            # halves the full-row/32-row array transition overhead.
            state = {}
            last_tiles = [None]

            def emit_quad(k):
                g0 = 2 * k
                pts = []
                tiles = []
                for j, g in enumerate((g0, g0 + 1)):
                    iw, gg = divmod(g, GPW)
                    if gg == 0:
                        state[iw] = {
                            "av": [
                                av_ps.tile([P, IW], F32, tag="avps",
                                           name=f"av{c}_{iw}")
                                for c in range(CH)
                            ],
                            "acc": accpool.tile(
                                [P, 2, IW], BF16, tag="acc", name=f"acc_{iw}"
                            ),
                        }
                    tiles.append(pe_ps.tile([P, 2, IW], F32, tag="peps",
                                            name=f"ps_e{g}"))
                for j, g in enumerate((g0, g0 + 1)):
                    iw, gg = divmod(g, GPW)
                    win = bass.ts(iw, IW)
                    for m in range(2):
                        jt = 2 * gg + m
                        r = 2 * j + m
                        nc.tensor.matmul(
                            tiles[j][:, m, :],
                            k4[r * C8 : (r + 1) * C8, bass.ts(jt, P)],
                            q4[r * C8 : (r + 1) * C8, win],
                            start=True, stop=True,
                            tile_position=(r * C8, 0),
                        )
                last_tiles[0] = tiles
                for j, g in enumerate((g0, g0 + 1)):
                    iw, gg = divmod(g, GPW)
                    pt = ptpool.tile([P, 2, IW], BF16, tag="pt", name=f"pt{g}")
                    nc.scalar.activation(
                        pt, tiles[j], mybir.ActivationFunctionType.Exp
                    )
                    acc = state[iw]["acc"]
                    if gg == 0:
                        nc.vector.tensor_copy(acc, pt)
                    else:
                        nc.vector.tensor_add(acc, acc, pt)
                    pts.append(pt)
                return pts

            def emit_av(g, pt):
                iw, gg = divmod(g, GPW)
                av = state[iw]["av"]
                for m in range(2):
                    jt = 2 * gg + m
                    for ch in range(CH):
                        nc.tensor.matmul(
                            av[ch],
                            vt[jt][:, bass.ts(ch, P)],
                            pt[:, m, :],
                            start=(gg == 0 and m == 0),
                            stop=(gg == GPW - 1 and m == 1),
                            skip_group_check=True,
                        )
                if gg == GPW - 1:
                    # free the av banks promptly: one copy on the DVE, one
                    # on the scalar engine so the two drain in parallel and
                    # the next window's AV matmuls start ~700ns sooner
                    av_sb = []
                    for ch in range(CH):
                        a_sb = outpool.tile([P, IW], F32, tag="osb",
                                            name=f"avsb{ch}_{iw}")
                        if ch == 0:
                            nc.vector.tensor_copy(a_sb, av[ch])
                        else:
                            nc.scalar.copy(a_sb, av[ch])
                        av_sb.append(a_sb)
                    state[iw]["av_sb"] = av_sb

            def emit_epilogue(iw):
                st = state.pop(iw)
                acc, av_sb = st["acc"], st["av_sb"]
                win = bass.ts(iw, IW)
                ps_r = last_tiles[0][0][:, 0, :]
                for s in range(2):
                    nc.tensor.matmul(
                        ps_r, ones_g, acc[:, s, :],
                        start=(s == 0), stop=(s == 1),
                    )
                rinv = smallwork.tile([P, IW], F32, tag="rinv", name=f"rinv{iw}")
                nc.vector.reciprocal_approx_fast(rinv, ps_r)
                xw = x_win(iw)
                for ch in range(CH):
                    o_sb = outpool.tile([P, IW], F32, tag="osb",
                                        name=f"osb{ch}_{iw}")
                    nc.vector.tensor_mul(o_sb, av_sb[ch], rinv)
                    nc.vector.scalar_tensor_tensor(
                        out=o_sb, in0=o_sb, scalar=gbv[:, ch : ch + 1],
                        in1=xw[:, ch, :],
                        op0=mybir.AluOpType.add, op1=mybir.AluOpType.add,
                    )
                    if ch == 0:
                        eng = nc.sync
                    elif iw == NW - 1:
                        eng = nc.scalar
                    else:
                        eng = nc.gpsimd
                    eng.dma_start(
                        out=out_d[ch * P : (ch + 1) * P, win], in_=o_sb
                    )

            NK = NG // 2
            prev = None
            for k in range(NK + 1):
                if k < NK:
                    if xc_late:
                        emit_xcast(xc_late.pop(0))
                    cur = emit_quad(k)
                if k > 0:
                    emit_av(2 * (k - 1), prev[0])
                    emit_av(2 * (k - 1) + 1, prev[1])
                if k < NK:
                    if qk_late:
                        emit_qkproj(qk_late.pop(0))
                    for _ in range(4):
                        if vjt_late:
                            emit_vproj(vjt_late.pop(0))
                    if k == 2:
                        emit_consts()
                    prev = cur
                g_done = 2 * (k - 1) + 1 if k > 0 else -1
                for w in range(NW):
                    if w in state and "av_sb" in state[w] and g_done >= 16 * w + 17:
                        emit_epilogue(w)
            for w in range(NW):
                if w in state:
                    emit_epilogue(w)

    nc.finalize()
    return nc


_NC_CACHE: dict[int, bass.Bass] = {}


def _get_nc(n: int) -> bass.Bass:
    if n not in _NC_CACHE:
        _NC_CACHE[n] = build_attention_nc(n)
    return _NC_CACHE[n]


def kernel(x, Wq, bq, Wk, bk, Wv, bv, gamma):
    B, c, h, w = x.shape
    n = h * w
    assert B == 8 and c == C
    nc = _get_nc(n)
    xf = np.ascontiguousarray(np.asarray(x, dtype=np.float32).reshape(B, c, n))
    common = {
        "Wq": np.ascontiguousarray(np.asarray(Wq, dtype=np.float32)),
        "bq": np.ascontiguousarray(np.asarray(bq, dtype=np.float32)),
        "Wk": np.ascontiguousarray(np.asarray(Wk, dtype=np.float32)),
        "bk": np.ascontiguousarray(np.asarray(bk, dtype=np.float32)),
        "Wv": np.ascontiguousarray(np.asarray(Wv, dtype=np.float32)),
        "bv": np.ascontiguousarray(np.asarray(bv, dtype=np.float32)),
        "gamma": np.ascontiguousarray(np.asarray(gamma, dtype=np.float32)),
    }
    in_maps = [{"x": xf[b], **common} for b in range(B)]
    res = run_bass_kernel_spmd(nc, in_maps, core_ids=list(range(B)))
    out = np.stack([res.results[b]["out"].reshape(c, h, w) for b in range(B)])
    return out.astype(np.float32)

